# revision 4
# baseline (speedup 1.0000x reference)
"""Trainium2 Bass kernel for DifferentiableTopK (Sinkhorn top-k masking).

Math (per batch row s in R^n, n=2048, K=256, eps=1e-3):
  reference builds log_P[i,j] = -(s_i - sorted(s)_j)^2/eps, runs 2 Sinkhorn
  normalization iterations (col then row), and returns logsumexp over the
  first K (sorted) columns.

This kernel works in the *sorted domain* where the matrix
  G[a,b] = exp(-1000*(x_a - x_b)^2)   (x = sorted scores, descending)
is symmetric, so every Sinkhorn reduction (column or row sums) becomes a
weighted ROW sum, computable as a TensorEngine matvec against the stored G
tiles. The log-domain normalizers never need to be materialized:

  S1 = G @ 1          w1 = 1/S1
  S2 = G @ w1         w2 = 1/S2
  S3 = G @ w2         w3 = 1/S3
  S4 = G @ w3
  M[a]   = 0 if a<K else -1000*(x_a - x_{K-1})^2          (stability shift)
  ET[b,a] = exp(-1000*(x_a-x_b)^2 - M[a])   for b<K
  Ksum   = ET^T @ w3[:K]
  out_sorted[a] = M[a] + log(Ksum[a] / S4[a])

out_sorted is permuted back to the original order on the host. Sharding is
pure data parallel: 32 batch rows -> 8 cores x 4 rows.
"""
import math
import sys

sys.path.insert(0, "/opt/trn_rl_repo")

import numpy as np
from contextlib import ExitStack

import concourse.bass as bass
import concourse.mybir as mybir
from concourse import bacc, tile
from concourse.bass_utils import run_bass_kernel_spmd

N = 2048
B = 32
NCORES = 8
BPC = B // NCORES  # batches per core
K = 256
NBLK = N // 128  # 16 partition blocks
SQRT1000 = float(np.float32(math.sqrt(1000.0)))
F32 = mybir.dt.float32
AF = mybir.ActivationFunctionType


def build_program():
    """Build + compile the per-core Bass program (same for all 8 cores)."""
    nc = bacc.Bacc("TRN2", target_bir_lowering=False, debug=False)

    d_xs = nc.dram_tensor("xs", [BPC, 1, N], F32, kind="ExternalInput").ap()
    d_mneg = nc.dram_tensor("mneg", [BPC, 1, N], F32, kind="ExternalInput").ap()
    d_sqb = nc.dram_tensor("sqbias", [BPC, 128, NBLK], F32, kind="ExternalInput").ap()
    d_mv = nc.dram_tensor("mvec", [BPC, 128, NBLK], F32, kind="ExternalInput").ap()
    d_out = nc.dram_tensor("out", [BPC, 128, NBLK], F32, kind="ExternalOutput").ap()

    with tile.TileContext(nc) as tc:
        with ExitStack() as ctx:
            gp = ctx.enter_context(tc.tile_pool(name="gpool", bufs=NBLK))
            etp = ctx.enter_context(tc.tile_pool(name="etpool", bufs=2))
            xbp = ctx.enter_context(tc.tile_pool(name="xbpool", bufs=1))
            rows = ctx.enter_context(tc.tile_pool(name="rows", bufs=3))
            tiny = ctx.enter_context(tc.tile_pool(name="tiny", bufs=2))
            cons = ctx.enter_context(tc.tile_pool(name="cons", bufs=1))
            pp = ctx.enter_context(tc.tile_pool(name="pp", bufs=2, space="PSUM"))
            dbp = ctx.enter_context(tc.tile_pool(name="dbounce", bufs=2, space="DRAM"))

            ones = cons.tile([1, 128], F32)
            nc.gpsimd.memset(ones[:], 1.0)

            def bounce_to_pm(psrow):
                """[1, N] psum row -> [128, NBLK] partition-major SBUF tile."""
                srow = rows.tile([1, N], F32, tag="row")
                nc.scalar.copy(srow[:], psrow)
                dt = dbp.tile([1, N], F32, tag="dscr")
                nc.sync.dma_start(dt[:], srow[:])
                pm = tiny.tile([128, NBLK], F32, tag="spm")
                nc.sync.dma_start(pm[:], dt[0].rearrange("(c p) -> p c", p=128))
                return pm

            for b in range(BPC):
                xrow = rows.tile([1, N], F32, tag="row")
                nc.sync.dma_start(xrow[:], d_xs[b])
                mrow = rows.tile([1, N], F32, tag="row")
                nc.sync.dma_start(mrow[:], d_mneg[b])
                sqb = tiny.tile([128, NBLK], F32, tag="sqb")
                nc.sync.dma_start(sqb[:], d_sqb[b])
                mv = tiny.tile([128, NBLK], F32, tag="mv")
                nc.sync.dma_start(mv[:], d_mv[b])

                # broadcast x across partitions: pxb[p, j] = x_j
                pxb = pp.tile([128, N], F32, tag="ps")
                for c in range(4):
                    cs = slice(c * 512, (c + 1) * 512)
                    nc.tensor.matmul(pxb[:, cs], ones[:], xrow[0:1, cs],
                                     start=True, stop=True)
                xb = xbp.tile([128, N], F32, tag="xb")
                nc.scalar.copy(xb[:], pxb[:])

                # broadcast -M across partitions (kept in PSUM, consumed by ET)
                pmr = pp.tile([128, N], F32, tag="ps")
                for c in range(4):
                    cs = slice(c * 512, (c + 1) * 512)
                    nc.tensor.matmul(pmr[:, cs], ones[:], mrow[0:1, cs],
                                     start=True, stop=True)

                # G build: G_m[p, j] = exp(-1000*(x_j - x_{m*128+p})^2)
                # accum_out of the Exp gives S1 (row sums) for free.
                s1 = tiny.tile([128, NBLK], F32, tag="s1")
                gt = []
                for m in range(NBLK):
                    g = gp.tile([128, N], F32, tag="g")
                    nc.scalar.activation(g[:], xb[:], AF.Square,
                                         bias=sqb[:, m:m + 1], scale=SQRT1000)
                    nc.scalar.activation(g[:], g[:], AF.Exp, scale=-1.0,
                                         accum_out=s1[:, m:m + 1])
                    gt.append(g)

                # ET blocks (b<K on partitions, all a on free dim):
                # ET[p, a] = exp(-1000*(x_a - x_{blk*128+p})^2 - M[a])
                et = []
                for blk in range(2):
                    e = etp.tile([128, N], F32, tag="et")
                    nc.scalar.activation(e[:], xb[:], AF.Square,
                                         bias=sqb[:, blk:blk + 1], scale=SQRT1000)
                    # e = (e * -1) + pmr  == -1000*d^2 - M
                    nc.vector.scalar_tensor_tensor(e[:], e[:], -1.0, pmr[:],
                                                   op0=mybir.AluOpType.mult,
                                                   op1=mybir.AluOpType.add)
                    nc.scalar.activation(e[:], e[:], AF.Exp, scale=1.0)
                    et.append(e)

                w1 = tiny.tile([128, NBLK], F32, tag="w")
                nc.vector.reciprocal(w1[:], s1[:])

                def s_pass(w):
                    ps = pp.tile([1, N], F32, tag="ps")
                    for c in range(4):
                        cs = slice(c * 512, (c + 1) * 512)
                        for m in range(NBLK):
                            nc.tensor.matmul(ps[0:1, cs], w[:, m:m + 1],
                                             gt[m][:, cs],
                                             start=(m == 0), stop=(m == NBLK - 1))
                    return bounce_to_pm(ps[0:1, :])

                s2 = s_pass(w1)
                w2 = tiny.tile([128, NBLK], F32, tag="w")
                nc.vector.reciprocal(w2[:], s2[:])
                s3 = s_pass(w2)
                w3 = tiny.tile([128, NBLK], F32, tag="w")
                nc.vector.reciprocal(w3[:], s3[:])
                s4 = s_pass(w3)

                # Ksum[a] = sum_{b<K} ET[b, a] * w3[b]
                pk = pp.tile([1, N], F32, tag="ps")
                for c in range(4):
                    cs = slice(c * 512, (c + 1) * 512)
                    for blk in range(2):
                        nc.tensor.matmul(pk[0:1, cs], w3[:, blk:blk + 1],
                                         et[blk][:, cs],
                                         start=(blk == 0), stop=(blk == 1))
                kpm = bounce_to_pm(pk[0:1, :])

                # out = M + ln(Ksum / S4)
                r4 = tiny.tile([128, NBLK], F32, tag="r4")
                nc.vector.reciprocal(r4[:], s4[:])
                q = tiny.tile([128, NBLK], F32, tag="q")
                nc.vector.tensor_mul(q[:], kpm[:], r4[:])
                lq = tiny.tile([128, NBLK], F32, tag="lq")
                nc.scalar.activation(lq[:], q[:], AF.Ln)
                osb = tiny.tile([128, NBLK], F32, tag="osb")
                nc.vector.tensor_add(osb[:], lq[:], mv[:])
                nc.sync.dma_start(d_out[b], osb[:])

    nc.compile()
    return nc


_NC_CACHE = None


def _get_program():
    global _NC_CACHE
    if _NC_CACHE is None:
        _NC_CACHE = build_program()
    return _NC_CACHE


def kernel(scores: np.ndarray) -> np.ndarray:
    scores = np.ascontiguousarray(np.asarray(scores, dtype=np.float32))
    assert scores.shape == (B, N), scores.shape

    # ---- host prep: sort + per-batch auxiliary arrays ----
    orders = np.argsort(-scores, axis=-1, kind="stable")  # descending
    xs = np.take_along_axis(scores, orders, axis=-1)  # [B, N] sorted desc

    d_tau = xs - xs[:, K - 1:K]  # x_a - x_{K-1}
    M = np.where(np.arange(N)[None, :] < K,
                 np.float32(0.0),
                 (np.float32(-1000.0) * d_tau * d_tau).astype(np.float32)
                 ).astype(np.float32)  # [B, N]
    sqbias = (np.float32(-SQRT1000) * xs).astype(np.float32)

    def pm(a):  # [B, N] -> [B, 128, NBLK] partition-major
        return np.ascontiguousarray(a.reshape(B, NBLK, 128).transpose(0, 2, 1))

    in_maps = []
    for c in range(NCORES):
        sl = slice(c * BPC, (c + 1) * BPC)
        in_maps.append({
            "xs": np.ascontiguousarray(xs[sl].reshape(BPC, 1, N)),
            "mneg": np.ascontiguousarray((-M[sl]).reshape(BPC, 1, N)),
            "sqbias": np.ascontiguousarray(pm(sqbias)[sl]),
            "mvec": np.ascontiguousarray(pm(M)[sl]),
        })

    nc = _get_program()
    res = run_bass_kernel_spmd(nc, in_maps, core_ids=list(range(NCORES)))

    out = np.empty((B, N), dtype=np.float32)
    for c in range(NCORES):
        o = res.results[c]["out"]  # [BPC, 128, NBLK]
        for b in range(BPC):
            gb = c * BPC + b
            out_sorted = np.ascontiguousarray(o[b].T).reshape(N)
            out[gb, orders[gb]] = out_sorted
    return out


if __name__ == "__main__":
    x = np.random.randn(B, N).astype(np.float32)
    y = kernel(x)
    print("kernel ran, out shape", y.shape, "finite:", np.isfinite(y).all())


# revision 8
# speedup vs baseline: 3.4576x; 3.4576x over previous
"""Trainium2 Bass kernel for DifferentiableTopK (Sinkhorn top-k masking).

Math (per batch row s in R^n, n=2048, K=256, eps=1e-3): the reference builds
log_P[i,j] = -(s_i - sorted(s)_j)^2/eps, runs 2 Sinkhorn normalizations
(col then row), and returns logsumexp over the first K (sorted) columns.

Kernel strategy (per batch, sorted domain, x = sorted scores descending):
  G[a,b] = exp(-1000*(x_a-x_b)^2) is symmetric, so all Sinkhorn reductions
  are weighted row sums = TensorEngine matvecs against stored G tiles:
    S1 = G @ 1 ; w1 = 1/S1 ; S2 = G @ w1 ; w2 = 1/S2 ; S3 = G @ w2
    w3 = 1/S3 ; S4 = G @ w3
    M[a] = 0 if a<K else -1000*(x_a - x_{K-1})^2
    ET[b,a] = exp(-1000*(x_a-x_b)^2 - M[a]) for b<K ; Ksum = ET^T @ w3[:K]
    out_sorted[a] = M[a] + log(Ksum[a] / S4[a])
  G is built on the TensorEngine as a K=2 matmul (outer-product expansion of
  the squared distance) + one ScalarEngine Exp; G is band-limited (entries
  with |x_a-x_b| > 0.3225 underflow to exactly 0 in fp32), so only chunks
  overlapping the band are built / multiplied. G is stored in bf16 (error
  class ~2e-3 absolute on outputs, invisible at the output scale of ~3e4).
  S1 falls out of the Exp's accum_out. Matvecs keep G stationary (128x128
  bf16 blocks) so results land partition-major — no transposes needed.

Sharding: pure data parallel, 32 rows -> 8 cores x 4. Host does the sort and
the tiny per-row prep; device does all n^2 work; host inverse-permutes.
"""
import math
import sys

sys.path.insert(0, "/opt/trn_rl_repo")

import numpy as np
from contextlib import ExitStack

import concourse.bass as bass
import concourse.mybir as mybir
from concourse import bacc, tile
from concourse.bass_utils import run_bass_kernel_spmd

N = 2048
B = 32
NCORES = 8
BPC = B // NCORES
K = 256
NBLK = N // 128   # 16 partition blocks
NCH = N // 512    # 4 build chunks
BAND = 0.3225     # exp(-1000*d^2) == 0 exactly in fp32 beyond this |d|
F32 = mybir.dt.float32
BF16 = mybir.dt.bfloat16
AF = mybir.ActivationFunctionType


def _coverage(xs_all):
    """Union (over all 32 rows) band coverage per batch slot.

    Returns (cov512, cov128): cov512[b][m] = sorted chunk ids (of 4) that
    block m needs; cov128[b][m] = sorted 128-blocks k with band overlap.
    Union over the 8 cores' rows sharing slot b keeps the single SPMD
    program correct for every core (skipped pairs are zero on all cores).
    """
    cov512 = [[set() for _ in range(NBLK)] for _ in range(BPC)]
    cov128 = [[set() for _ in range(NBLK)] for _ in range(BPC)]
    for row in range(B):
        b = row % BPC
        x = xs_all[row]
        bhi = [x[m * 128] for m in range(NBLK)]
        blo = [x[m * 128 + 127] for m in range(NBLK)]
        for m in range(NBLK):
            for kb in range(NBLK):
                if not (blo[m] - bhi[kb] > BAND or blo[kb] - bhi[m] > BAND):
                    cov128[b][m].add(kb)
            for c in range(NCH):
                chi, clo = x[c * 512], x[c * 512 + 511]
                if not (blo[m] - chi > BAND or clo - bhi[m] > BAND):
                    cov512[b][m].add(c)
    return ([[sorted(s) for s in row] for row in cov512],
            [[sorted(s) for s in row] for row in cov128])


def build_program(cov512, cov128):
    nc = bacc.Bacc("TRN2", target_bir_lowering=False, debug=False)

    d_lhs3 = nc.dram_tensor("lhs3", [BPC, 3, N], F32, kind="ExternalInput").ap()
    d_rhs3 = nc.dram_tensor("rhs3", [BPC, 3, N], F32, kind="ExternalInput").ap()
    d_eb = nc.dram_tensor("ebias", [BPC, 128, NBLK], F32, kind="ExternalInput").ap()
    d_mv = nc.dram_tensor("mvec", [BPC, 128, NBLK], F32, kind="ExternalInput").ap()
    d_out = nc.dram_tensor("out", [BPC, 128, NBLK], F32, kind="ExternalOutput").ap()

    with tile.TileContext(nc) as tc:
        with ExitStack() as ctx:
            gp = ctx.enter_context(tc.tile_pool(name="gpool", bufs=2 * NBLK))
            etp = ctx.enter_context(tc.tile_pool(name="etpool", bufs=4))
            rows = ctx.enter_context(tc.tile_pool(name="rows", bufs=2))
            tiny = ctx.enter_context(tc.tile_pool(name="tiny", bufs=2))
            acc = ctx.enter_context(tc.tile_pool(name="acc", bufs=2))
            cons = ctx.enter_context(tc.tile_pool(name="cons", bufs=1))
            pb = ctx.enter_context(tc.tile_pool(name="pbuild", bufs=4, space="PSUM"))
            pv = ctx.enter_context(tc.tile_pool(name="pvec", bufs=2, space="PSUM"))

            onescol = cons.tile([128, 1], BF16)
            nc.gpsimd.memset(onescol[:], 1.0)

            for b in range(BPC):
                lhs3 = rows.tile([3, N], F32, tag="lhs3")
                nc.sync.dma_start(lhs3[:], d_lhs3[b])
                rhs3 = rows.tile([3, N], F32, tag="rhs3")
                nc.sync.dma_start(rhs3[:], d_rhs3[b])
                eb = tiny.tile([128, NBLK], F32, tag="eb")
                nc.sync.dma_start(eb[:], d_eb[b])
                mv = tiny.tile([128, NBLK], F32, tag="mv")
                nc.sync.dma_start(mv[:], d_mv[b])

                # ---- G build (banded): psum = 2000 x_a x_b - 1000 x_b^2,
                #      G = exp(psum - 1000 x_a^2) in bf16, accum -> S1 parts
                s1acc = acc.tile([128, NBLK * NCH], F32, tag="s1acc")
                nc.gpsimd.memset(s1acc[:], 0.0)
                gt = []
                for m in range(NBLK):
                    g = gp.tile([128, N], BF16, tag="g")
                    for c in cov512[b][m]:
                        cs = slice(c * 512, (c + 1) * 512)
                        ps = pb.tile([128, 512], F32, tag="pb")
                        nc.tensor.matmul(ps[:], lhs3[0:2, m * 128:(m + 1) * 128],
                                         rhs3[0:2, cs], start=True, stop=True)
                        nc.scalar.activation(
                            g[:, cs], ps[:], AF.Exp, bias=eb[:, m:m + 1],
                            scale=1.0,
                            accum_out=s1acc[:, m * NCH + c:m * NCH + c + 1])
                    gt.append(g)

                # ---- ET build (full width): psum = 2000 x_a x_b - 1000 x_a^2
                #      - M[a] (K=3), ET = exp(psum - 1000 x_b^2) in bf16
                et = []
                for blk in range(2):
                    e = etp.tile([128, N], BF16, tag="et")
                    for c in range(NCH):
                        cs = slice(c * 512, (c + 1) * 512)
                        ps = pb.tile([128, 512], F32, tag="pb")
                        nc.tensor.matmul(ps[:], lhs3[0:3, blk * 128:(blk + 1) * 128],
                                         rhs3[0:3, cs], start=True, stop=True)
                        nc.scalar.activation(e[:, cs], ps[:], AF.Exp,
                                             bias=eb[:, blk:blk + 1], scale=1.0)
                    et.append(e)

                # S1 = sum of per-chunk accums
                s1 = tiny.tile([128, NBLK], F32, tag="s")
                nc.vector.tensor_reduce(
                    s1[:], s1acc[:].rearrange("p (m c) -> p m c", c=NCH),
                    axis=mybir.AxisListType.X, op=mybir.AluOpType.add)

                def recip_cast(s):
                    wf = tiny.tile([128, NBLK], F32, tag="wf")
                    nc.vector.reciprocal(wf[:], s[:])
                    wb = tiny.tile([128, NBLK], BF16, tag="wb")
                    nc.vector.tensor_copy(wb[:], wf[:])
                    return wb

                def matvec(wb):
                    """S[m-block] = sum_k G_k[:, m-block].T @ w_k  (banded)."""
                    ps = pv.tile([128, NBLK], F32, tag="pv")
                    for m in range(NBLK):
                        ks = cov128[b][m]
                        for i, kb in enumerate(ks):
                            nc.tensor.matmul(
                                ps[:, m:m + 1],
                                gt[kb][:, m * 128:(m + 1) * 128],
                                wb[:, kb:kb + 1],
                                start=(i == 0), stop=(i == len(ks) - 1))
                    return ps

                w1 = recip_cast(s1)
                ps2 = matvec(w1)
                w2 = recip_cast(ps2)
                ps3 = matvec(w2)
                w3 = recip_cast(ps3)
                ps4 = matvec(w3)

                # Ksum[m-block] = sum_{blk<2} ET_blk[:, m-block].T @ w3_blk
                pk = pv.tile([128, NBLK], F32, tag="pv")
                for m in range(NBLK):
                    for blk in range(2):
                        nc.tensor.matmul(pk[:, m:m + 1],
                                         et[blk][:, m * 128:(m + 1) * 128],
                                         w3[:, blk:blk + 1],
                                         start=(blk == 0), stop=(blk == 1))

                # out = M + ln(Ksum / S4)
                r4 = tiny.tile([128, NBLK], F32, tag="r4")
                nc.vector.reciprocal(r4[:], ps4[:])
                q = tiny.tile([128, NBLK], F32, tag="q")
                nc.vector.tensor_mul(q[:], pk[:], r4[:])
                lq = tiny.tile([128, NBLK], F32, tag="lq")
                nc.scalar.activation(lq[:], q[:], AF.Ln)
                osb = tiny.tile([128, NBLK], F32, tag="osb")
                nc.vector.tensor_add(osb[:], lq[:], mv[:])
                nc.sync.dma_start(d_out[b], osb[:])

    nc.compile()
    return nc


_CACHE = {}


def prepare(scores: np.ndarray):
    """Host prep: sort, coverage, program build, per-core input maps."""
    scores = np.ascontiguousarray(np.asarray(scores, dtype=np.float32))
    assert scores.shape == (B, N), scores.shape

    orders = np.argsort(-scores, axis=-1, kind="stable")
    xs = np.take_along_axis(scores, orders, axis=-1)  # [B, N] sorted desc

    cov512, cov128 = _coverage(xs)
    key = (xs.tobytes(),)
    if key not in _CACHE:
        _CACHE.clear()
        _CACHE[key] = build_program(cov512, cov128)
    nc = _CACHE[key]

    d_tau = xs - xs[:, K - 1:K]
    M = np.where(np.arange(N)[None, :] < K, np.float32(0.0),
                 (np.float32(-1000.0) * d_tau * d_tau).astype(np.float32)
                 ).astype(np.float32)
    ones = np.ones_like(xs)
    lhs3 = np.stack([xs, ones, ones], axis=1).astype(np.float32)  # [B,3,N]
    rhs3 = np.stack([np.float32(2000.0) * xs,
                     np.float32(-1000.0) * xs * xs,
                     -M], axis=1).astype(np.float32)
    ebias = (np.float32(-1000.0) * xs * xs).astype(np.float32)

    def pm(a):
        return np.ascontiguousarray(a.reshape(B, NBLK, 128).transpose(0, 2, 1))

    eb_pm, mv_pm = pm(ebias), pm(M)
    in_maps = []
    for c in range(NCORES):
        sl = slice(c * BPC, (c + 1) * BPC)
        in_maps.append({
            "lhs3": np.ascontiguousarray(lhs3[sl]),
            "rhs3": np.ascontiguousarray(rhs3[sl]),
            "ebias": np.ascontiguousarray(eb_pm[sl]),
            "mvec": np.ascontiguousarray(mv_pm[sl]),
        })
    return nc, in_maps, orders


def postprocess(results, orders):
    out = np.empty((B, N), dtype=np.float32)
    for c in range(NCORES):
        o = results[c]["out"]  # [BPC, 128, NBLK]
        for b in range(BPC):
            gb = c * BPC + b
            out[gb, orders[gb]] = np.ascontiguousarray(o[b].T).reshape(N)
    return out


def kernel(scores: np.ndarray) -> np.ndarray:
    nc, in_maps, orders = prepare(scores)
    res = run_bass_kernel_spmd(nc, in_maps, core_ids=list(range(NCORES)))
    return postprocess(res.results, orders)


if __name__ == "__main__":
    x = np.random.randn(B, N).astype(np.float32)
    y = kernel(x)
    print("kernel ran, out shape", y.shape, "finite:", np.isfinite(y).all())


# revision 9
# speedup vs baseline: 6.5004x; 1.8801x over previous
"""Trainium2 Bass kernel for DifferentiableTopK (Sinkhorn top-k masking).

Math (per batch row s in R^n, n=2048, K=256, eps=1e-3): the reference builds
log_P[i,j] = -(s_i - sorted(s)_j)^2/eps, runs 2 Sinkhorn normalizations
(col then row), and returns logsumexp over the first K (sorted) columns.

Kernel strategy (per batch, sorted domain, x = sorted scores descending):
  G[a,b] = exp(-1000*(x_a-x_b)^2) is symmetric, so all Sinkhorn reductions
  are weighted row sums = TensorEngine matvecs against stored G tiles:
    S1 = G @ 1 ; w1 = 1/S1 ; S2 = G @ w1 ; w2 = 1/S2 ; S3 = G @ w2
    w3 = 1/S3 ; S4 = G @ w3
    M[a] = 0 if a<K else -1000*(x_a - x_{K-1})^2
    ET[b,a] = exp(-1000*(x_a-x_b)^2 - M[a]) for b<K ; Ksum = ET^T @ w3[:K]
    out_sorted[a] = M[a] + log(Ksum[a] / S4[a])

  G/ET are built on the TensorEngine as an outer-product expansion of the
  squared distance: t0 = x_a*(2000 x_b) + (-1000 x_b^2) (+ (-M[a]) for ET),
  with every factor split into 3 bf16 limbs so a single-pass bf16 matmul
  (K=9 for G, K=12 for ET) reproduces fp32-level accuracy; one ScalarEngine
  Exp (bias = -1000 x_a^2, the natural_log_exp_and_others table) finishes
  each tile in bf16. All work is band-limited: entries with
  |x_a - x_b| > 0.3225 underflow to exactly 0 in fp32 and are skipped.
  S1 falls out of the Exp's accum_out. Matvecs keep G stationary (128x128
  bf16 blocks) so results land partition-major in PSUM — no transposes.

Sharding: pure data parallel, 32 rows -> 8 cores x 4. Host does the sort and
tiny per-row prep; device does all n^2 work; host inverse-permutes.
"""
import math
import sys

sys.path.insert(0, "/opt/trn_rl_repo")

import numpy as np
import ml_dtypes
from contextlib import ExitStack

import concourse.bass as bass
import concourse.mybir as mybir
from concourse import bacc, tile
from concourse.bass_utils import run_bass_kernel_spmd

N = 2048
B = 32
NCORES = 8
BPC = B // NCORES
K = 256
NBLK = N // 128   # 16 partition blocks
NCH = N // 512    # 4 build chunks
BAND = 0.3225     # exp(-1000*d^2) == 0 exactly in fp32 beyond this |d|
ETLIM = 104.0     # exp(-t) == 0 exactly in fp32 for t > 104
F32 = mybir.dt.float32
BF16 = mybir.dt.bfloat16
AF = mybir.ActivationFunctionType
BF = ml_dtypes.bfloat16


def _coverage(xs_all):
    """Union (over all 32 rows) band coverage per batch slot.

    cov512[b][m]: build chunks (of 4) needed for G block m.
    cov128[b][m]: contraction 128-blocks k for the S matvecs.
    etch[b][blk]: build chunks needed for ET block blk (b<K rows).
    etmv[b][m]:   ET blocks blk contributing to Ksum output block m.
    """
    cov512 = [[set() for _ in range(NBLK)] for _ in range(BPC)]
    cov128 = [[set() for _ in range(NBLK)] for _ in range(BPC)]
    etch = [[set() for _ in range(2)] for _ in range(BPC)]
    etmv = [[set() for _ in range(NBLK)] for _ in range(BPC)]
    for row in range(B):
        b = row % BPC
        x = xs_all[row].astype(np.float64)
        M = np.where(np.arange(N) < K, 0.0, 1000.0 * (x - x[K - 1]) ** 2)
        bhi = [x[m * 128] for m in range(NBLK)]
        blo = [x[m * 128 + 127] for m in range(NBLK)]
        for m in range(NBLK):
            for kb in range(NBLK):
                if not (blo[m] - bhi[kb] > BAND or blo[kb] - bhi[m] > BAND):
                    cov128[b][m].add(kb)
            for c in range(NCH):
                chi, clo = x[c * 512], x[c * 512 + 511]
                if not (blo[m] - chi > BAND or clo - bhi[m] > BAND):
                    cov512[b][m].add(c)
        # ET: entry (bb, a) alive iff 1000*(x_a-x_bb)^2 + M[a] <= ETLIM
        for blk in range(2):
            xb = x[blk * 128:(blk + 1) * 128]
            lo_b, hi_b = xb[-1], xb[0]
            # min over bb in block of (x_a - x_bb)^2 = interval distance
            gap = np.maximum(np.maximum(lo_b - x, x - hi_b), 0.0)
            alive = 1000.0 * gap * gap + M <= ETLIM
            for c in range(NCH):
                if alive[c * 512:(c + 1) * 512].any():
                    etch[b][blk].add(c)
            for m in range(NBLK):
                if alive[m * 128:(m + 1) * 128].any():
                    etmv[b][m].add(blk)
    srt = lambda ll: [[sorted(s) for s in row] for row in ll]
    return srt(cov512), srt(cov128), srt(etch), srt(etmv)


def build_program(cov512, cov128, etch, etmv):
    nc = bacc.Bacc("TRN2", target_bir_lowering=False, debug=False)

    d_lhs = nc.dram_tensor("lhsb", [BPC, 12, N], BF16, kind="ExternalInput").ap()
    d_rhs = nc.dram_tensor("rhsb", [BPC, 12, N], BF16, kind="ExternalInput").ap()
    d_eb = nc.dram_tensor("ebias", [BPC, 128, NBLK], F32, kind="ExternalInput").ap()
    d_mv = nc.dram_tensor("mvec", [BPC, 128, NBLK], F32, kind="ExternalInput").ap()
    d_out = nc.dram_tensor("out", [BPC, 128, NBLK], F32, kind="ExternalOutput").ap()
    d_warm = nc.dram_tensor("warm", [128, 1], F32, kind="ExternalOutput").ap()

    with tile.TileContext(nc) as tc:
        with ExitStack() as ctx:
            gp = ctx.enter_context(tc.tile_pool(name="gpool", bufs=2 * NBLK))
            etp = ctx.enter_context(tc.tile_pool(name="etpool", bufs=4))
            rows = ctx.enter_context(tc.tile_pool(name="rows", bufs=2))
            tiny = ctx.enter_context(tc.tile_pool(name="tiny", bufs=2))
            acc = ctx.enter_context(tc.tile_pool(name="acc", bufs=2))
            pb = ctx.enter_context(tc.tile_pool(name="pbuild", bufs=4, space="PSUM"))
            pv = ctx.enter_context(tc.tile_pool(name="pvec", bufs=2, space="PSUM"))

            # HAM warm-up: ~10us of dense matmul so the PE clock-gate opens
            # (4/8 -> 8/8) before the real work starts.
            lhs0 = rows.tile([12, N], BF16, tag="lhsb")
            nc.sync.dma_start(lhs0[:], d_lhs[0])
            rhs0 = rows.tile([12, N], BF16, tag="rhsb")
            nc.sync.dma_start(rhs0[:], d_rhs[0])
            wsum = tiny.tile([128, 1], F32, tag="warm")
            for i in range(24):
                pw = pb.tile([128, 512], F32, tag="pb")
                nc.tensor.matmul(pw[:], lhs0[0:9, 0:128], rhs0[0:9, 0:512],
                                 start=True, stop=True)
                if i == 23:
                    nc.vector.tensor_reduce(wsum[:], pw[:],
                                            axis=mybir.AxisListType.X,
                                            op=mybir.AluOpType.add)
            nc.sync.dma_start(d_warm[:], wsum[:])

            for b in range(BPC):
                if b == 0:
                    lhsb, rhsb = lhs0, rhs0
                else:
                    lhsb = rows.tile([12, N], BF16, tag="lhsb")
                    nc.sync.dma_start(lhsb[:], d_lhs[b])
                    rhsb = rows.tile([12, N], BF16, tag="rhsb")
                    nc.sync.dma_start(rhsb[:], d_rhs[b])
                eb = tiny.tile([128, NBLK], F32, tag="eb")
                nc.sync.dma_start(eb[:], d_eb[b])
                mv = tiny.tile([128, NBLK], F32, tag="mv")
                nc.sync.dma_start(mv[:], d_mv[b])

                # ---- G build (banded, bf16 limbs K=9):
                # psum = x_a*(2000 x_b) - 1000 x_b^2 ; G = exp(psum - 1000 x_a^2)
                s1acc = acc.tile([128, NBLK * NCH], F32, tag="s1acc")
                nc.gpsimd.memset(s1acc[:], 0.0)
                gt = []
                for m in range(NBLK):
                    g = gp.tile([128, N], BF16, tag="g")
                    for c in cov512[b][m]:
                        cs = slice(c * 512, (c + 1) * 512)
                        ps = pb.tile([128, 512], F32, tag="pb")
                        nc.tensor.matmul(ps[:], lhsb[0:9, m * 128:(m + 1) * 128],
                                         rhsb[0:9, cs], start=True, stop=True)
                        nc.scalar.activation(
                            g[:, cs], ps[:], AF.Exp, bias=eb[:, m:m + 1],
                            scale=1.0,
                            accum_out=s1acc[:, m * NCH + c:m * NCH + c + 1])
                    gt.append(g)

                # ---- ET build (banded, K=12 adds the -M[a] limbs):
                et = []
                for blk in range(2):
                    e = etp.tile([128, N], BF16, tag="et")
                    for c in etch[b][blk]:
                        cs = slice(c * 512, (c + 1) * 512)
                        ps = pb.tile([128, 512], F32, tag="pb")
                        nc.tensor.matmul(ps[:], lhsb[0:12, blk * 128:(blk + 1) * 128],
                                         rhsb[0:12, cs], start=True, stop=True)
                        nc.scalar.activation(e[:, cs], ps[:], AF.Exp,
                                             bias=eb[:, blk:blk + 1], scale=1.0)
                    et.append(e)

                # S1 = sum of per-chunk accums
                s1 = tiny.tile([128, NBLK], F32, tag="s")
                nc.vector.tensor_reduce(
                    s1[:], s1acc[:].rearrange("p (m c) -> p m c", c=NCH),
                    axis=mybir.AxisListType.X, op=mybir.AluOpType.add)

                def recip_cast(s):
                    wf = tiny.tile([128, NBLK], F32, tag="wf")
                    nc.vector.reciprocal(wf[:], s[:])
                    wb = tiny.tile([128, NBLK], BF16, tag="wb")
                    nc.vector.tensor_copy(wb[:], wf[:])
                    return wb

                def matvec(wb):
                    """S[m-block] = sum_k G_k[:, m-block].T @ w_k  (banded)."""
                    ps = pv.tile([128, NBLK], F32, tag="pv")
                    for m in range(NBLK):
                        ks = cov128[b][m]
                        for i, kb in enumerate(ks):
                            nc.tensor.matmul(
                                ps[:, m:m + 1],
                                gt[kb][:, m * 128:(m + 1) * 128],
                                wb[:, kb:kb + 1],
                                start=(i == 0), stop=(i == len(ks) - 1))
                    return ps

                w1 = recip_cast(s1)
                ps2 = matvec(w1)
                w2 = recip_cast(ps2)
                ps3 = matvec(w2)
                w3 = recip_cast(ps3)
                ps4 = matvec(w3)

                # Ksum[m-block] = sum_blk ET_blk[:, m-block].T @ w3_blk (banded)
                pk = pv.tile([128, NBLK], F32, tag="pv")
                for m in range(NBLK):
                    bs = etmv[b][m]
                    for i, blk in enumerate(bs):
                        nc.tensor.matmul(pk[:, m:m + 1],
                                         et[blk][:, m * 128:(m + 1) * 128],
                                         w3[:, blk:blk + 1],
                                         start=(i == 0), stop=(i == len(bs) - 1))

                # out = M + ln(Ksum / S4)
                r4 = tiny.tile([128, NBLK], F32, tag="r4")
                nc.vector.reciprocal(r4[:], ps4[:])
                q = tiny.tile([128, NBLK], F32, tag="q")
                nc.vector.tensor_mul(q[:], pk[:], r4[:])
                lq = tiny.tile([128, NBLK], F32, tag="lq")
                nc.scalar.activation(lq[:], q[:], AF.Ln)
                osb = tiny.tile([128, NBLK], F32, tag="osb")
                nc.vector.tensor_add(osb[:], lq[:], mv[:])
                nc.sync.dma_start(d_out[b], osb[:])

    nc.compile()
    return nc


_CACHE = {}


def _limbs3(v):
    """Split fp32 array into 3 bf16 limbs (exact to ~2^-27 relative)."""
    v = v.astype(np.float32)
    l0 = v.astype(BF)
    r = v - l0.astype(np.float32)
    l1 = r.astype(BF)
    l2 = (r - l1.astype(np.float32)).astype(BF)
    return l0, l1, l2


def prepare(scores: np.ndarray):
    """Host prep: sort, coverage, program build, per-core input maps."""
    scores = np.ascontiguousarray(np.asarray(scores, dtype=np.float32))
    assert scores.shape == (B, N), scores.shape

    orders = np.argsort(-scores, axis=-1, kind="stable")
    xs = np.take_along_axis(scores, orders, axis=-1)  # [B, N] sorted desc

    covs = _coverage(xs)
    key = (xs.tobytes(),)
    if key not in _CACHE:
        _CACHE.clear()
        _CACHE[key] = build_program(*covs)
    nc = _CACHE[key]

    d_tau = xs - xs[:, K - 1:K]
    M = np.where(np.arange(N)[None, :] < K, np.float32(0.0),
                 (np.float32(-1000.0) * d_tau * d_tau).astype(np.float32)
                 ).astype(np.float32)

    a0, a1, a2 = _limbs3(xs)
    c0, c1, c2 = _limbs3(np.float32(2000.0) * xs)
    dd0, dd1, dd2 = _limbs3(np.float32(-1000.0) * xs * xs)
    m0, m1, m2 = _limbs3(-M)
    one = np.ones_like(xs).astype(BF)
    # K rows pair lhs[k] with rhs[k]; products a_i*c_j kept for i+j<=2.
    lhsb = np.stack([a0, a0, a0, a1, a1, a2, one, one, one, one, one, one],
                    axis=1)  # [B, 12, N] bf16
    rhsb = np.stack([c0, c1, c2, c0, c1, c0, dd0, dd1, dd2, m0, m1, m2],
                    axis=1)
    ebias = (np.float32(-1000.0) * xs * xs).astype(np.float32)

    def pm(a):
        return np.ascontiguousarray(a.reshape(B, NBLK, 128).transpose(0, 2, 1))

    eb_pm, mv_pm = pm(ebias), pm(M)
    in_maps = []
    for c in range(NCORES):
        sl = slice(c * BPC, (c + 1) * BPC)
        in_maps.append({
            "lhsb": np.ascontiguousarray(lhsb[sl]),
            "rhsb": np.ascontiguousarray(rhsb[sl]),
            "ebias": np.ascontiguousarray(eb_pm[sl]),
            "mvec": np.ascontiguousarray(mv_pm[sl]),
        })
    return nc, in_maps, orders


def postprocess(results, orders):
    out = np.empty((B, N), dtype=np.float32)
    for c in range(NCORES):
        o = results[c]["out"]  # [BPC, 128, NBLK]
        for b in range(BPC):
            gb = c * BPC + b
            out[gb, orders[gb]] = np.ascontiguousarray(o[b].T).reshape(N)
    return out


def kernel(scores: np.ndarray) -> np.ndarray:
    nc, in_maps, orders = prepare(scores)
    res = run_bass_kernel_spmd(nc, in_maps, core_ids=list(range(NCORES)))
    return postprocess(res.results, orders)


if __name__ == "__main__":
    x = np.random.randn(B, N).astype(np.float32)
    y = kernel(x)
    print("kernel ran, out shape", y.shape, "finite:", np.isfinite(y).all())
